# revision 28
# baseline (speedup 1.0000x reference)
"""KA-attention (crossinf) Trainium2 kernel.

Math notes (exact, not approximations):
  reference computes  out = softmax_j( sum_d sigmoid(y_q)[b,h,i,d] + sum_d sigmoid(y_k)[b,h,j,d] )
  The first term is constant along the softmax axis j, so it cancels
  (softmax shift-invariance):  out[b,h,i,j] = softmax_j( B[b,h,j] ),
  B[b,h,j] = sum_d sigmoid(y_k)[b,h,j,d],  y_k = f_q * scale_sp + silu(qf) @ Wq.T.
  Only the q-path (base_weight_q, coef_q) is mathematically needed.

Sharding: tensor-shard the 8192 output rows of base_weight_q across 8 cores
(1024 rows / core).  Weights ship as fp8-e4m3 (8 MB/core = the HBM roofline,
~23 us at 358 GB/s); the matmul runs in fp8 DoubleRow mode (2 k-tiles per
instruction, f32 PSUM accumulate).  The KAN sin-spline is computed on all 128
partitions via the Chebyshev recurrence sin(kr) = 2cos(r)sin((k-1)r) -
sin((k-2)r) (one range reduction instead of 8), then PE-transposed back to
the [batch, n] layout of the matmul accumulator.  Host applies the (tiny)
softmax and broadcasts over the cancelled i axis.
"""

import sys
import numpy as np
import ml_dtypes

for _p in ("/opt/trn_rl_repo", "/root/.axon_site/_ro/trn_rl_repo"):
    if _p not in sys.path:
        sys.path.append(_p)

import concourse.bass as bass
import concourse.tile as tile
from concourse import bacc, mybir
from concourse.bass_utils import run_bass_kernel_spmd

# Problem shapes (hardcoded per contract)
B, H, P, D = 16, 4, 128, 16
NUM = H * P * D          # 8192
NF = 8                   # spline basis size
NC = 8                   # cores
NS = NUM // NC           # 1024 output rows per core
KT = NUM // 128          # 64 k-tiles of 128
NH = NS // 2             # 512: output rows per half
NG = 8                   # weight DMA groups per half (8 k-tiles each)
F32 = mybir.dt.float32
BF16 = mybir.dt.bfloat16
F8 = mybir.dt.float8e4
NP_F8 = ml_dtypes.float8_e4m3
NP_BF16 = ml_dtypes.bfloat16

# knobs (test.py pokes these)
TRACE = False
TRACE_KW = {}
DEBUG_Y = False          # adds a y=[16,1024] debug output to the program
USE_DR = True            # fp8 DoubleRow matmul perf mode

_CACHE = {}


def _build_bass(debug_y):
    nc = bacc.Bacc("TRN2", target_bir_lowering=False, debug=False)
    # [half, g, p, s, n] = Wq[n0 + half*512 + n, (g*8+s)*128 + p]
    wt = nc.declare_dram_parameter("wt", [2, NG, 128, 8, NH], F8, isOutput=False)
    # [p, kt, b] = qf[b, kt*128 + p]
    qt = nc.declare_dram_parameter("qt", [128, KT, B], BF16, isOutput=False)
    # [p, j, b] = qf[b, n0 + j*128 + p]   (per-core shard of qf, f32)
    q2 = nc.declare_dram_parameter("q2", [128, 8, B], F32, isOutput=False)
    # [p, f, j, b] = coef_q[n0 + j*128 + p, f] * scale_sp[n0 + j*128 + p]
    cfb = nc.declare_dram_parameter("cfb", [128, NF, 8, B], BF16, isOutput=False)
    bs = nc.declare_dram_parameter("bs", [B, NS // D], F32, isOutput=True)
    if debug_y:
        yd = nc.declare_dram_parameter("yd", [B, NS], F32, isOutput=True)

    act = mybir.ActivationFunctionType
    mm = mybir.AluOpType
    TWO_PI = 6.283185307179586
    PI_BOUND = 3.141592653589793
    PI_CLAMP = 3.1415925

    with tile.TileContext(nc) as tc:
        with (
            tc.tile_pool(name="w", bufs=16) as wpool,
            tc.tile_pool(name="stat", bufs=1) as stat,
            tc.tile_pool(name="psum", bufs=1, space=bass.MemorySpace.PSUM) as psum,
        ):
            # ---- input DMAs, all on the sync HWDGE ring.  The weight
            # stream (16 x 512KB, ~23 us) is the critical path: start it
            # immediately and interleave the small inputs into its head.
            # The first matmul is gated by the ACT table load (~9 us)
            # anyway, so chunks landing before sq8 is ready cost nothing.
            qt_s = stat.tile([128, KT, B], BF16)
            q2_s = stat.tile([128, 8, B], F32)
            cfb_s = stat.tile([128, NF, 8, B], BF16)
            idm_s = stat.tile([128, 128], F32)

            w_t = [[None] * NG for _ in range(2)]
            for h in range(2):
                for g in range(NG):
                    w_t[h][g] = wpool.tile([128, 8, NH], F8, tag="w",
                                           name=f"w{h}_{g}")

            def wdma(h, g):
                nc.sync.dma_start(out=w_t[h][g], in_=wt[h, g])

            wdma(0, 0)
            nc.sync.dma_start(out=qt_s, in_=qt[:, :, :])
            wdma(0, 1)
            nc.sync.dma_start(out=q2_s, in_=q2[:, :, :])
            nc.sync.dma_start(out=cfb_s, in_=cfb[:, :, :, :])
            for g in range(2, NG):
                wdma(0, g)
            for g in range(NG):
                wdma(1, g)
            # identity for the PE transposes, built on the idle gpsimd
            from concourse.masks import make_identity
            make_identity(nc, idm_s)

            # ---- silu(qf) -> fp8 lhsT for the matmul.  ACT does only
            # Sigmoid/Copy in this kernel (single table set, no ~2.7 us
            # table switches): silu = x * sigmoid(x) via one DVE multiply.
            sg_t = stat.tile([128, KT, B], F32)
            silu_f = stat.tile([128, KT, B], F32)
            sq8 = stat.tile([128, KT, B], F8)
            nc.scalar.activation(sg_t, qt_s, act.Sigmoid)
            nc.vector.tensor_mul(silu_f, qt_s, sg_t)
            nc.vector.tensor_copy(out=sq8, in_=silu_f)

            # ---- KAN sin spline on all 128 partitions.
            # x = q2; one range reduction r = x - 2pi*round-ish via single
            # wrap (|x| < 3pi for randn inputs), then DVE Horner polys for
            # sin(r) and sin(r/2) (no ACT Sin -> no table switch), then
            # Chebyshev: s_k = 2cos(r)*s_{k-1} - s_{k-2}; sin(k*x)==sin(k*r).
            r = stat.tile([128, 8, B], F32)
            u = stat.tile([128, 8, B], F32)
            hp = stat.tile([128, 8, B], F32)
            sh = stat.tile([128, 8, B], F32)
            sh2 = stat.tile([128, 8, B], F32)
            c2t = stat.tile([128, 8, B], F32)
            s_k = [stat.tile([128, 8, B], F32, name=f"sk{k}") for k in range(NF)]
            prod = stat.tile([128, 8, B], F32)
            sp2 = stat.tile([128, 8, B], F32)

            # sin(x)=x*P(x^2), sin(x/2)=x*R(x^2) on [-pi,pi] (max err ~1e-5)
            CP = [0.9999944957316066, -0.1666412673321153, 0.008314162875924815,
                  -0.00019324412734932634, 2.1707342353035046e-06]
            CR = [0.4999999969159194, -0.020833319183121486, 0.0002604060779512913,
                  -1.5472848400529542e-06, 5.072627221664558e-09]

            nc.vector.add_range_wrap(r, q2_s, 0.0, PI_BOUND, TWO_PI)
            nc.vector.tensor_mul(u, r, r)

            def horner(out, coeffs):
                # out = r * (c0 + c1 u + ... c4 u^4), via hp chain
                # (no clamp needed: the polynomial is smooth past +-pi)
                nc.vector.tensor_scalar(hp, u, coeffs[4], coeffs[3],
                                        op0=mm.mult, op1=mm.add)
                for c in (coeffs[2], coeffs[1], coeffs[0]):
                    nc.vector.tensor_mul(hp, hp, u)
                    nc.vector.tensor_scalar_add(hp, hp, c)
                nc.vector.tensor_mul(out, hp, r)

            horner(s_k[0], CP)                                 # sin(r)
            horner(sh, CR)                                     # sin(r/2)
            nc.vector.tensor_mul(sh2, sh, sh)
            # 2cos(r) = 2 - 4 sin^2(r/2)
            nc.vector.tensor_scalar(c2t, sh2, -4.0, 2.0,
                                    op0=mm.mult, op1=mm.add)
            nc.vector.tensor_mul(s_k[1], c2t, s_k[0])          # sin(2r)
            for k in range(2, NF):
                nc.vector.tensor_mul(s_k[k], c2t, s_k[k - 1])
                nc.vector.tensor_sub(s_k[k], s_k[k], s_k[k - 2])
            nc.vector.tensor_mul(sp2, cfb_s[:, 0], s_k[0])
            for k in range(1, NF):
                nc.vector.tensor_mul(prod, cfb_s[:, k], s_k[k])
                nc.vector.tensor_add(sp2, sp2, prod)

            # ---- transpose spline back to [b, n] layout (PE identity trick)
            spT = psum.tile([B, NS], F32)
            for j in range(8):
                nc.tensor.transpose(spT[:, j * 128:(j + 1) * 128],
                                    sp2[:, j, :], idm_s)
            # stage in SBUF: the tail add may read only one PSUM operand
            spS = stat.tile([B, NS], F32)
            nc.scalar.activation(spS, spT, act.Copy)

            # ---- base matmul: acc[b, n] = sum_k silu(qf)[b,k] * Wq[n0+n, k]
            # fp8 DoubleRow: each instruction consumes 2 k-tiles.
            acc = [psum.tile([B, NH], F32, name=f"acc{h}") for h in range(2)]
            kstep = 2 if USE_DR else 1
            pm = mybir.MatmulPerfMode.DoubleRow if USE_DR else None
            for h in range(2):
                for g in range(NG):
                    for j in range(0, 8, kstep):
                        kt0 = g * 8 + j
                        nc.tensor.matmul(
                            acc[h][:, :],
                            sq8[:, kt0:kt0 + kstep, :],
                            w_t[h][g][:, j:j + kstep, :],
                            start=(g == 0 and j == 0),
                            stop=(g == NG - 1 and j == 8 - kstep),
                            perf_mode=pm,
                        )

            # ---- tail per half: y = base + spline; B = sum_d sigmoid(y)
            y_s = stat.tile([B, NS], F32)
            bsum = stat.tile([B, NS // D], F32)
            for h in range(2):
                sig_h = stat.tile([B, NH], F32, tag=f"sig{h}")
                nc.vector.tensor_add(y_s[:, h * NH:(h + 1) * NH], acc[h][:, :],
                                     spS[:, h * NH:(h + 1) * NH])
                nc.scalar.activation(sig_h, y_s[:, h * NH:(h + 1) * NH],
                                     act.Sigmoid)
                nc.vector.reduce_sum(
                    out=bsum[:, h * 32:(h + 1) * 32],
                    in_=sig_h.rearrange("p (i d) -> p i d", d=D),
                    axis=mybir.AxisListType.X,
                )
            nc.sync.dma_start(out=bs[:, :], in_=bsum)
            if debug_y:
                nc.scalar.dma_start(out=yd[:, :], in_=y_s)
    nc.compile()
    return nc


def kernel(q, k, v, grid, base_weight_q, base_weight_k, coef_q, coef_k, scale_sp):
    q = np.asarray(q, dtype=np.float32)
    base_weight_q = np.asarray(base_weight_q, dtype=np.float32)
    coef_q = np.asarray(coef_q, dtype=np.float32)
    scale_sp = np.asarray(scale_sp, dtype=np.float32)

    qf = q.reshape(B, NUM)
    # lhsT layout: (128, KT, B) with [kp, kt, b] = qf[b, kt*128 + kp]
    qtf = np.ascontiguousarray(qf.T.reshape(KT, 128, B).transpose(1, 0, 2))
    qt = qtf.astype(NP_BF16)
    idm = np.eye(128, dtype=np.float32)
    w8 = base_weight_q.astype(NP_F8)
    cs = coef_q * scale_sp[:, None]   # fold scale into the spline coefs

    in_maps = []
    for c in range(NC):
        n0 = c * NS
        # [half, g, p, s, n] = w8[n0 + half*512 + n, (g*8+s)*128 + p]
        wt = np.ascontiguousarray(
            w8[n0:n0 + NS, :].T                  # [k, n]
            .reshape(NG, 8, 128, 2, NH)          # [g, s, p, half, n]
            .transpose(3, 0, 2, 1, 4))           # [half, g, p, s, n]
        q2 = np.ascontiguousarray(qtf[:, c * 8:(c + 1) * 8, :])
        # [p, f, j, b] = cs[n0 + j*128 + p, f]
        cfb = np.ascontiguousarray(np.broadcast_to(
            cs[n0:n0 + NS, :].reshape(8, 128, 1, NF, 1)
            .transpose(1, 3, 0, 2, 4),           # [p, f, j, 1, 1]
            (128, NF, 8, 1, B)).reshape(128, NF, 8, B)).astype(NP_BF16)
        in_maps.append({"wt": wt, "qt": qt, "q2": q2, "cfb": cfb})

    key = ("nc", DEBUG_Y, USE_DR)
    if key not in _CACHE:
        _CACHE[key] = _build_bass(DEBUG_Y)
    res = run_bass_kernel_spmd(_CACHE[key], in_maps, list(range(NC)),
                               trace=TRACE, **TRACE_KW)
    _CACHE["last_result"] = res

    Bmat = np.empty((B, H, P), np.float32)
    for c in range(NC):
        h, j0 = c // 2, 64 * (c % 2)
        Bmat[:, h, j0:j0 + 64] = res.results[c]["bs"]

    # softmax over j (float32, same stabilized form jax uses)
    m = Bmat.max(axis=-1, keepdims=True)
    e = np.exp(Bmat - m)
    soft = (e / e.sum(axis=-1, keepdims=True)).astype(np.float32)
    return np.ascontiguousarray(
        np.broadcast_to(soft[:, :, None, :], (B, H, P, P)))
